# revision 9
# baseline (speedup 1.0000x reference)
"""ConvChunk2d patch-extraction kernel for Trainium2 (8 NeuronCores).

Reference computes, for x of shape (8, 64, 128, 128):
    out[n, y*128 + xx, c, a, b] = xpad[n, (a*192 + b*64 + c) // 9, y + a, xx + b]
with xpad zero-padded by 1 on H/W, output shape (8*16384, 64, 3, 3).

Pure data movement (gather + replication), memory-bound.  Strategy:
data-parallel over batch (1 image per core).  Per core:
  - Load input as A0[y_partition, ch, x+1] (x zero-padded in the free dim),
    plus partition-shifted copies Am (row y-1) / Ap (row y+1) built with
    SBUF->SBUF DMAs, so the kernel row-shift never crosses partitions in a
    compute op.
  - For p = 3a+b and s in [0,9): the output columns j = c*9 + p with
    c = 9*ch + s - 64p form an affine family over ch, so one strided
    tensor_copy per (p, s) moves all of them (81 copies per x-block),
    spread across Vector/Scalar/GPSIMD engines.
  - Output tiles (128 rows y, XB*576 floats) DMA out as large contiguous
    runs per partition.
"""

import math

import numpy as np

import concourse.bacc as bacc
import concourse.bass as bass
import concourse.mybir as mybir
from concourse.bass_utils import run_bass_kernel_spmd
from concourse.tile import TileContext

N, C, H, W = 8, 64, 128, 128
K = 3
L = H * W
J = C * K * K  # 576 output columns per spatial location
XB = 16  # x-block width; out tile = [128, XB*J] floats
NBLK = W // XB
F32 = mybir.dt.float32


def _jobs():
    """(a, b, ch_lo, cnt, c0, p) for each affine copy family."""
    jobs = []
    for p in range(K * K):
        a, b = divmod(p, K)
        for s in range(9):
            ch_lo = math.ceil((64 * p - s) / 9)
            ch_hi = (63 + 64 * p - s) // 9
            cnt = ch_hi - ch_lo + 1
            c0 = 9 * ch_lo + s - 64 * p
            jobs.append((a, b, ch_lo, cnt, c0, p))
    return jobs


def build_nc():
    nc = bacc.Bacc("TRN2")
    x = nc.declare_dram_parameter("x", [C, H, W], F32, isOutput=False)
    out = nc.declare_dram_parameter("out", [L, J], F32, isOutput=True)

    with TileContext(nc) as tc:
        with (
            tc.tile_pool(name="a", bufs=1) as apool,
            tc.tile_pool(name="t", bufs=2) as tpool,
        ):
            A0 = apool.tile([128, C, W + 2], F32, tag="a0")
            Am = apool.tile([128, C, W + 2], F32, tag="am")
            Ap = apool.tile([128, C, W + 2], F32, tag="ap")

            # Zero-pad columns x=0 and x=W+1 of all three tiles.
            for Ak in (A0, Am, Ap):
                nc.vector.memset(Ak[:, :, 0:1], 0.0)
                nc.vector.memset(Ak[:, :, W + 1 : W + 2], 0.0)
            # Boundary rows must be zero padding: Am[0] = Ap[127] = 0.
            # Compute-engine partition bases must be quadrant-aligned, so row
            # 127 can't be memset directly; instead zero the whole last
            # quadrant of Ap BEFORE the loads overwrite rows 96..126.
            nc.gpsimd.memset(Am[0:1, :, :], 0.0)
            nc.vector.memset(Ap[96:128, :, :], 0.0)
            # Load x[ch, y, xx] -> A0[y, ch, xx+1], plus partition-shifted
            # copies Am[y] = row y-1, Ap[y] = row y+1, straight from HBM.
            # One 2D DMA per (channel, tile): 3D-AP DMAs serialize onto one
            # SDMA engine, and partition-shifted SBUF->SBUF runs at ~15 GB/s
            # on a single engine either way; per-partition-descriptor 2D
            # HBM->SBUF loads spray across all 16 engines at line rate.
            for ch in range(C):
                nc.sync.dma_start(out=A0[:, ch, 1 : W + 1], in_=x[ch, :, :])
                nc.sync.dma_start(out=Am[1:128, ch, 1 : W + 1], in_=x[ch, 0:127, :])
                nc.sync.dma_start(out=Ap[0:127, ch, 1 : W + 1], in_=x[ch, 1:128, :])

            jobs = _jobs()
            outr = out[:, :].rearrange("(y xx) j -> y xx j", xx=W)
            # Greedy engine balancing with measured per-copy cost models (ns):
            # DVE ~ 75 + (58+e)/0.96, ACT ~ (224+e)/1.2, GPSIMD ~ 360 + 1.22e.
            load = [0.0, 0.0, 0.0]
            for blk in range(NBLK):
                x0 = blk * XB
                T = tpool.tile([128, XB, C, K * K], F32, tag="t")
                for a, b, ch_lo, cnt, c0, p in jobs:
                    Ak = (Am, A0, Ap)[a]
                    dst = T[:, :, c0 : c0 + 9 * (cnt - 1) + 1 : 9, p].transpose([0, 2, 1])
                    src = Ak[:, ch_lo : ch_lo + cnt, x0 + b : x0 + b + XB]
                    e = cnt * XB
                    costs = (75 + (58 + e) / 0.96, (224 + e) / 1.2, 360 + 1.22 * e)
                    eng = min(range(3), key=lambda i: load[i] + costs[i])
                    load[eng] += costs[eng]
                    if eng == 0:
                        nc.vector.tensor_copy(dst, src)
                    elif eng == 1:
                        nc.scalar.copy(dst, src)
                    else:
                        nc.gpsimd.tensor_copy(dst, src)
                nc.sync.dma_start(
                    out=outr[:, x0 : x0 + XB, :],
                    in_=T[:, :, :, :].rearrange("pp xx c q -> pp xx (c q)"),
                )
    nc.finalize()
    return nc


def kernel(x):
    x = np.ascontiguousarray(np.asarray(x, dtype=np.float32))
    assert x.shape == (N, C, H, W), x.shape
    nc = build_nc()
    in_maps = [{"x": x[n]} for n in range(N)]
    res = run_bass_kernel_spmd(nc, in_maps, list(range(N)))
    outs = [np.asarray(res.results[i]["out"]).reshape(L, C, K, K) for i in range(N)]
    return np.concatenate(outs, axis=0)


# revision 10
# speedup vs baseline: 2.1622x; 2.1622x over previous
"""ConvChunk2d patch-extraction kernel for Trainium2 (8 NeuronCores).

Reference computes, for x of shape (8, 64, 128, 128):
    out[n, y*128 + xx, c, a, b] = xpad[n, (a*192 + b*64 + c) // 9, y + a, xx + b]
with xpad zero-padded by 1 on H/W, output shape (8*16384, 64, 3, 3).

Pure data movement (gather + replication), memory-bound.  Strategy:
data-parallel over batch (1 image per core).  Per core:
  - Load input as A0[y_partition, ch, x+1] (x zero-padded in the free dim),
    plus partition-shifted copies Am (row y-1) / Ap (row y+1) built with
    SBUF->SBUF DMAs, so the kernel row-shift never crosses partitions in a
    compute op.
  - For p = 3a+b and s in [0,9): the output columns j = c*9 + p with
    c = 9*ch + s - 64p form an affine family over ch, so one strided
    tensor_copy per (p, s) moves all of them (81 copies per x-block),
    spread across Vector/Scalar/GPSIMD engines.
  - Output tiles (128 rows y, XB*576 floats) DMA out as large contiguous
    runs per partition.
"""

import math

import numpy as np

import concourse.bacc as bacc
import concourse.bass as bass
import concourse.mybir as mybir
from concourse.bass_utils import run_bass_kernel_spmd
from concourse.tile import TileContext

N, C, H, W = 8, 64, 128, 128
K = 3
L = H * W
J = C * K * K  # 576 output columns per spatial location
XB = 16  # x-block width; out tile = [128, XB*J] floats
NBLK = W // XB
F32 = mybir.dt.float32


def _jobs():
    """(a, b, ch_lo, cnt, c0, p) for each affine copy family."""
    jobs = []
    for p in range(K * K):
        a, b = divmod(p, K)
        for s in range(9):
            ch_lo = math.ceil((64 * p - s) / 9)
            ch_hi = (63 + 64 * p - s) // 9
            cnt = ch_hi - ch_lo + 1
            c0 = 9 * ch_lo + s - 64 * p
            jobs.append((a, b, ch_lo, cnt, c0, p))
    return jobs


def build_nc():
    nc = bacc.Bacc("TRN2")
    x = nc.declare_dram_parameter("x", [C, H, W], F32, isOutput=False)
    out = nc.declare_dram_parameter("out", [L, J], F32, isOutput=True)

    with TileContext(nc) as tc:
        with (
            tc.tile_pool(name="a", bufs=1) as apool,
            tc.tile_pool(name="t", bufs=2) as tpool,
        ):
            A0 = apool.tile([128, C, W + 2], F32, tag="a0")
            Am = apool.tile([128, C, W + 2], F32, tag="am")
            Ap = apool.tile([128, C, W + 2], F32, tag="ap")

            # Zero-pad columns x=0 and x=W+1 of all three tiles.
            for Ak in (A0, Am, Ap):
                nc.vector.memset(Ak[:, :, 0:1], 0.0)
                nc.vector.memset(Ak[:, :, W + 1 : W + 2], 0.0)
            # Load x[ch, y, xx] -> A0[y, ch, xx+1], plus partition-shifted
            # copies Am[y] = row y-1, Ap[y] = row y+1, straight from HBM.
            # Constraints discovered on HW:
            #  - Only 2D-AP, FULL-128-partition HWDGE DMAs get split across
            #    the 16 SDMA engines; 3D APs or 127-partition dests
            #    serialize onto engine 0 (~25 ns/descriptor, one engine).
            #  - So the shifted loads wrap the HBM source by one row into
            #    the neighboring channel (flat view) to keep 128 partitions;
            #    the garbage row lands in a partition that is zeroed after.
            xf = x[:, :, :].rearrange("c h w -> (c h) w")
            for ch in range(C):
                nc.sync.dma_start(out=A0[:, ch, 1 : W + 1], in_=x[ch, :, :])
                if ch > 0:
                    nc.sync.dma_start(
                        out=Am[:, ch, 1 : W + 1],
                        in_=xf[ch * H - 1 : ch * H + 127, :],
                    )
                else:
                    nc.sync.dma_start(
                        out=Am[1:128, 0, 1 : W + 1], in_=x[0, 0:127, :]
                    )
                if ch < C - 1:
                    nc.sync.dma_start(
                        out=Ap[:, ch, 1 : W + 1],
                        in_=xf[ch * H + 1 : ch * H + 129, :],
                    )
                else:
                    nc.sync.dma_start(
                        out=Ap[0:127, C - 1, 1 : W + 1], in_=x[C - 1, 1:128, :]
                    )
            # Boundary rows must be zero padding: Am[0] = Ap[127] = 0.
            # (After the loads — the wrapped loads scribble on them.)
            # Compute-engine partition bases must be quadrant-aligned, so
            # row 127 can't be memset; DMA the zeroed Am row 0 into it.
            nc.gpsimd.memset(Am[0:1, :, :], 0.0)
            nc.sync.dma_start(out=Ap[127:128, :, :], in_=Am[0:1, :, :])

            jobs = _jobs()
            outr = out[:, :].rearrange("(y xx) j -> y xx j", xx=W)
            # Greedy engine balancing with measured per-copy cost models (ns):
            # DVE ~ 75 + (58+e)/0.96, ACT ~ (224+e)/1.2, GPSIMD ~ 360 + 1.22e.
            load = [0.0, 0.0, 0.0]
            for blk in range(NBLK):
                x0 = blk * XB
                T = tpool.tile([128, XB, C, K * K], F32, tag="t")
                for a, b, ch_lo, cnt, c0, p in jobs:
                    Ak = (Am, A0, Ap)[a]
                    dst = T[:, :, c0 : c0 + 9 * (cnt - 1) + 1 : 9, p].transpose([0, 2, 1])
                    src = Ak[:, ch_lo : ch_lo + cnt, x0 + b : x0 + b + XB]
                    e = cnt * XB
                    costs = (75 + (58 + e) / 0.96, (224 + e) / 1.2, 360 + 1.22 * e)
                    eng = min(range(3), key=lambda i: load[i] + costs[i])
                    load[eng] += costs[eng]
                    if eng == 0:
                        nc.vector.tensor_copy(dst, src)
                    elif eng == 1:
                        nc.scalar.copy(dst, src)
                    else:
                        nc.gpsimd.tensor_copy(dst, src)
                nc.sync.dma_start(
                    out=outr[:, x0 : x0 + XB, :],
                    in_=T[:, :, :, :].rearrange("pp xx c q -> pp xx (c q)"),
                )
    nc.finalize()
    return nc


def kernel(x):
    x = np.ascontiguousarray(np.asarray(x, dtype=np.float32))
    assert x.shape == (N, C, H, W), x.shape
    nc = build_nc()
    in_maps = [{"x": x[n]} for n in range(N)]
    res = run_bass_kernel_spmd(nc, in_maps, list(range(N)))
    outs = [np.asarray(res.results[i]["out"]).reshape(L, C, K, K) for i in range(N)]
    return np.concatenate(outs, axis=0)


# revision 12
# speedup vs baseline: 3.3407x; 1.5450x over previous
"""ConvChunk2d patch-extraction kernel for Trainium2 (8 NeuronCores).

Reference computes, for x of shape (8, 64, 128, 128):
    out[n, y*128 + xx, c, a, b] = xpad[n, (a*192 + b*64 + c) // 9, y + a, xx + b]
with xpad zero-padded by 1 on H/W, output shape (8*16384, 64, 3, 3).

Pure data movement (gather + replication), memory-bound.  Strategy:
data-parallel over batch (1 image per core).  Per core:
  - Load input as A0[y_partition, ch, x+1] (x zero-padded in the free dim),
    plus partition-shifted copies Am (row y-1) / Ap (row y+1) built with
    SBUF->SBUF DMAs, so the kernel row-shift never crosses partitions in a
    compute op.
  - For p = 3a+b and s in [0,9): the output columns j = c*9 + p with
    c = 9*ch + s - 64p form an affine family over ch, so one strided
    tensor_copy per (p, s) moves all of them (81 copies per x-block),
    spread across Vector/Scalar/GPSIMD engines.
  - Output tiles (128 rows y, XB*576 floats) DMA out as large contiguous
    runs per partition.
"""

import math

import numpy as np

import concourse.bacc as bacc
import concourse.bass as bass
import concourse.mybir as mybir
from concourse.bass_utils import run_bass_kernel_spmd
from concourse.tile import TileContext

N, C, H, W = 8, 64, 128, 128
K = 3
L = H * W
J = C * K * K  # 576 output columns per spatial location
XB = 16  # x-block width; out tile = [128, XB*J] floats
NBLK = W // XB
F32 = mybir.dt.float32


def _jobs():
    """(a, b, ch_lo, cnt, c0, p) for each affine copy family."""
    jobs = []
    for p in range(K * K):
        a, b = divmod(p, K)
        for s in range(9):
            ch_lo = math.ceil((64 * p - s) / 9)
            ch_hi = (63 + 64 * p - s) // 9
            cnt = ch_hi - ch_lo + 1
            c0 = 9 * ch_lo + s - 64 * p
            jobs.append((a, b, ch_lo, cnt, c0, p))
    return jobs


def build_nc():
    nc = bacc.Bacc("TRN2")
    x = nc.declare_dram_parameter("x", [C, H, W], F32, isOutput=False)
    out = nc.declare_dram_parameter("out", [L, J], F32, isOutput=True)

    with TileContext(nc) as tc:
        with (
            tc.tile_pool(name="a", bufs=1) as apool,
            tc.tile_pool(name="t", bufs=2) as tpool,
        ):
            A0 = apool.tile([128, C, W + 2], F32, tag="a0")
            Am = apool.tile([128, C, W + 2], F32, tag="am")
            Ap = apool.tile([128, C, W + 2], F32, tag="ap")

            # Zero-pad columns x=0 and x=W+1 of all three tiles.
            for Ak in (A0, Am, Ap):
                nc.vector.memset(Ak[:, :, 0:1], 0.0)
                nc.vector.memset(Ak[:, :, W + 1 : W + 2], 0.0)
            # Load x[ch, y, xx] -> A0[y, ch, xx+1], plus partition-shifted
            # copies Am[y] = row y-1, Ap[y] = row y+1, straight from HBM.
            # Constraints discovered on HW:
            #  - Only 2D-AP, FULL-128-partition HWDGE DMAs get split across
            #    the 16 SDMA engines; 3D APs or 127-partition dests
            #    serialize onto engine 0 (~25 ns/descriptor, one engine).
            #  - So the shifted loads wrap the HBM source by one row into
            #    the neighboring channel (flat view) to keep 128 partitions;
            #    the garbage row lands in a partition that is zeroed after.
            nc.sync.dma_start(
                out=A0[:, :, 1 : W + 1], in_=x[:, :, :].transpose([1, 0, 2])
            )
            # Bulk shifted loads keep 128 partitions (anything else
            # serializes onto one SDMA engine) by wrapping the flat HBM row
            # index by one row: partition 0 of Am (and 127 of Ap) receives a
            # garbage row from the adjacent channel, fixed up below.  The
            # channel at the tensor edge (ch=0 for Am, ch=63 for Ap) cannot
            # wrap in-bounds, so it loads as a small 127-partition DMA.
            sm = x[1:C, :, :].transpose([1, 0, 2])
            sm.offset -= W  # (y, ch, xx) -> row y-1 of ch, for ch in [1, C)
            nc.sync.dma_start(out=Am[:, 1:C, 1 : W + 1], in_=sm)
            nc.sync.dma_start(out=Am[1:128, 0, 1 : W + 1], in_=x[0, 0 : H - 1, :])
            sp = x[0 : C - 1, :, :].transpose([1, 0, 2])
            sp.offset += W  # row y+1 of ch, for ch in [0, C-1)
            nc.sync.dma_start(out=Ap[:, 0 : C - 1, 1 : W + 1], in_=sp)
            nc.sync.dma_start(out=Ap[0:127, C - 1, 1 : W + 1], in_=x[C - 1, 1:H, :])
            # Boundary rows must be zero padding: Am[0] = Ap[127] = 0.
            # (After the loads — the wrapped loads scribble on them.)
            # Compute-engine partition bases must be quadrant-aligned, so
            # row 127 can't be memset; DMA the zeroed Am row 0 into it.
            nc.gpsimd.memset(Am[0:1, :, :], 0.0)
            nc.sync.dma_start(out=Ap[127:128, :, :], in_=Am[0:1, :, :])

            jobs = _jobs()
            outr = out[:, :].rearrange("(y xx) j -> y xx j", xx=W)
            # Greedy engine balancing with measured per-copy cost models (ns):
            # DVE ~ 75 + (58+e)/0.96, ACT ~ (224+e)/1.2, GPSIMD ~ 360 + 1.22e.
            load = [0.0, 0.0, 0.0]
            for blk in range(NBLK):
                x0 = blk * XB
                T = tpool.tile([128, XB, C, K * K], F32, tag="t")
                for a, b, ch_lo, cnt, c0, p in jobs:
                    Ak = (Am, A0, Ap)[a]
                    dst = T[:, :, c0 : c0 + 9 * (cnt - 1) + 1 : 9, p].transpose([0, 2, 1])
                    src = Ak[:, ch_lo : ch_lo + cnt, x0 + b : x0 + b + XB]
                    e = cnt * XB
                    costs = (75 + (58 + e) / 0.96, (224 + e) / 1.2, 360 + 1.22 * e)
                    eng = min(range(3), key=lambda i: load[i] + costs[i])
                    load[eng] += costs[eng]
                    if eng == 0:
                        nc.vector.tensor_copy(dst, src)
                    elif eng == 1:
                        nc.scalar.copy(dst, src)
                    else:
                        nc.gpsimd.tensor_copy(dst, src)
                nc.sync.dma_start(
                    out=outr[:, x0 : x0 + XB, :],
                    in_=T[:, :, :, :].rearrange("pp xx c q -> pp xx (c q)"),
                )
    nc.finalize()
    return nc


def kernel(x):
    x = np.ascontiguousarray(np.asarray(x, dtype=np.float32))
    assert x.shape == (N, C, H, W), x.shape
    nc = build_nc()
    in_maps = [{"x": x[n]} for n in range(N)]
    res = run_bass_kernel_spmd(nc, in_maps, list(range(N)))
    outs = [np.asarray(res.results[i]["out"]).reshape(L, C, K, K) for i in range(N)]
    return np.concatenate(outs, axis=0)
